# revision 23
# baseline (speedup 1.0000x reference)
"""Trainium2 Bass kernel for nn_BasicBlock (binary-conv residual block).

Math (reference):
  h  = BN3( RPReLU1(BN1(bconv(sign(x), w1))) + x )
  out= BN4( RPReLU2(BN2(bconv(sign(h), w2))) + h )
with training-mode BN over the FULL batch (exact cross-device stats),
bconv = conv3x3(pad=1) with weights sign(w)*mean(|w|) per out-channel.

Strategy: data-parallel over batch on 8 NeuronCores (16 images/core).
 - Host ships x pre-padded to 30x30 (conv input), x unpadded (residual
   reads stay contiguous), and weights pre-transposed to the DoubleRow
   [i, ih, k, o] layout (layout-only prep; all math on device).
 - fp8e4 +-1 activations/weights; conv = 9 shifted DoubleRow matmuls
   into PSUM; integer sums exact. AF.Sign(0)=0 makes the padded zeros
   self-padding in the fp8 conv input buffer (no big memset; the BN3
   re-sign only rewrites interiors so pads stay 0 for conv2).
 - The h path stays fp32 end to end (the BN3 re-sign is flip-sensitive
   to rounding): S/h' tiles are fp32 in SBUF, h' round-trips through
   DRAM while conv2 reuses the tile slots, with the swap DMAs scheduled
   into SP-idle windows.
 - Exact BN stats via per-channel-half (sum,sumsq) AllGather + local
   add; per-half stagger hides half the AGs under conv matmuls; AG
   round-trip DMAs ride the ACT HWDGE queue so bulk SP traffic never
   delays them.
 - BN3 folded into the re-sign (sign(c3*h+d3)); 1/c3 folded into BN2
   coefs and normalized away by BN4 (c3>0).
 - Junk-matmul PE warmers bridge idle stretches (DVFS).
"""

import sys

import numpy as np

sys.path.insert(0, "/opt/trn_rl_repo")

from contextlib import ExitStack

import concourse.bacc as bacc
import concourse.bass as bass
import concourse.bass_utils as _bu
import concourse.mybir as mybir
import concourse.tile as tile

dt = mybir.dt
AF = mybir.ActivationFunctionType
ALU = mybir.AluOpType
AX = mybir.AxisListType

C = 256
H = W = 28
PH = PW = 30
SP = PH * PW          # padded pixels / image
HW = H * W            # valid pixels / image
MARG = 32             # margin around padded free axis (shifts up to +-31)
EPS = 1e-5
NPAR = 12
PJ = dict(g1=0, b1=1, g2=2, b2=3, g3=4, b3=5, g4=6, b4=7,
          gamma1=8, beta1=9, gamma2=10, beta2=11)


def _off(d):
    kh, kw = d // 3, d % 3
    return (kh - 1) * PW + (kw - 1)


def build_nc(n_img, n_cores):
    nc = bacc.Bacc("TRN2", target_bir_lowering=False, num_devices=n_cores,
                   name="basicblock")
    px_d = nc.declare_dram_parameter("px", [n_img, C, SP], dt.float32,
                                     isOutput=False)
    xu_d = nc.declare_dram_parameter("xu", [n_img, C, HW], dt.float32,
                                     isOutput=False)
    wt_d = nc.declare_dram_parameter("wt", [2, 128, 2 * 9 * C], dt.bfloat16,
                                     isOutput=False)
    p_d = nc.declare_dram_parameter("pars", [NPAR, C], dt.float32,
                                    isOutput=False)
    o_d = nc.declare_dram_parameter("out", [n_img, C, HW], dt.float16,
                                    isOutput=True)

    FREE = n_img * SP
    XBW = FREE + 2 * MARG
    NLOC = float(n_img * HW)
    NTOT = float(n_cores * n_img * HW)
    rg = [list(range(n_cores))]

    with ExitStack() as ctx:
        tc = ctx.enter_context(tile.TileContext(nc))
        sing = ctx.enter_context(tc.tile_pool(name="sing", bufs=1))
        wtp = ctx.enter_context(tc.tile_pool(name="wtp", bufs=2))
        stp = ctx.enter_context(tc.tile_pool(name="stp", bufs=1))
        dccp = ctx.enter_context(tc.tile_pool(name="dccp", bufs=1,
                                              space="DRAM"))
        dswp = ctx.enter_context(tc.tile_pool(name="dswp", bufs=2 * n_img,
                                              space="DRAM"))

        # ---- constants / params ----------------------------------------
        par = sing.tile([128, NPAR, 2], dt.float32, name="par")
        nc.sync.dma_start(out=par, in_=p_d[:, :].rearrange("j (h c) -> c j h",
                                                           h=2))
        epst = sing.tile([128, 1], dt.float32, name="epst")
        nc.vector.memset(epst, EPS)

        def P(j, ch):
            return par[:, PJ[j], ch:ch + 1]

        def cf(name, w=1):
            return stp.tile([128, w], dt.float32, name=name, tag=name)

        # ---- warmup collective (first cc pays ~50us init; trigger ASAP,
        # read the result only later so no DMA queue blocks on the init).
        diw = dccp.tile([256], dt.float32, name="diw", tag="diw")
        dow = dccp.tile([n_cores * 256], dt.float32, name="dow", tag="dow")
        win = sing.tile([128, 2], dt.float32, name="win")
        nc.vector.memset(win, 1.0)
        nc.sync.dma_start(out=diw.rearrange("(c f) -> c f", f=2), in_=win)
        nc.gpsimd.collective_compute(
            "AllGather", ALU.bypass, replica_groups=rg, ins=[diw], outs=[dow])

        def read_warm():
            warmt = cf("warmt", 2)
            nc.sync.dma_start(
                out=warmt, in_=dow.rearrange("(r x) -> x r", x=256)[0:128, 0:2])

        # ---- weight prep (staging pools scoped: freed before big pools) -
        # wt: [128(i), (2(k-half) 9(tap) 256(o))] fp8 per conv
        wt = {cv: wtp.tile([128, 2 * 9 * C], dt.float8e4, name=f"wt{cv}",
                           tag="wt") for cv in (1, 2)}

        def wt_ap(cv, d, m):
            return (wt[cv].rearrange("p (h k o) -> p h k o", h=2, o=C)
                    [:, :, d, m * 128:(m + 1) * 128])

        # alpha = mean|w| cancels exactly through training-mode BN (up to
        # an eps/alpha^2 perturbation ~1e-6 rel) so it is never computed.
        # Weights ship as bf16: rounding never flips sign, so wt is exact.
        with ExitStack() as wctx:
            wfp = wctx.enter_context(tc.tile_pool(name="wfp", bufs=2))
            for cv in (1, 2):
                wtv = wt[cv].rearrange("p (h x) -> p h x", h=2)
                for ih in (0, 1):
                    wf = wfp.tile([128, 9 * C], dt.bfloat16,
                                  name=f"wf{cv}{ih}", tag="wf")
                    nc.sync.dma_start(
                        out=wf,
                        in_=wt_d[cv - 1].rearrange("p (h x) -> p h x",
                                                   h=2)[:, ih])
                    nc.scalar.activation(wtv[:, ih], wf, AF.Sign)

        # ---- big pools (allocated after weight staging is released) ----
        xbp = ctx.enter_context(tc.tile_pool(name="xbp", bufs=1))
        s1p = ctx.enter_context(tc.tile_pool(name="s1p", bufs=2 * n_img))
        chkp = ctx.enter_context(tc.tile_pool(name="chkp", bufs=4))
        otp = ctx.enter_context(tc.tile_pool(name="otp", bufs=3))
        tmpp = ctx.enter_context(tc.tile_pool(name="tmpp", bufs=3))
        psp = ctx.enter_context(tc.tile_pool(name="psp", bufs=8, space="PSUM"))

        # xb: [128, 2(k-half), XBW] fp8, DoubleRow-interleaved conv input.
        xbt = xbp.tile([128, 2, XBW], dt.float8e4, name="xbt", tag="xb")
        for chh in (0, 1):
            nc.vector.memset(xbt[:, chh, 0:MARG], 0.0)
            nc.vector.memset(xbt[:, chh, MARG + FREE:], 0.0)

        # ---- PE clock warmers ------------------------------------------
        jw = sing.tile([128, 2, 128], dt.float8e4, name="jw")
        nc.vector.memset(jw, 0.0)
        jx = sing.tile([128, 2, 450], dt.float8e4, name="jx")
        nc.vector.memset(jx, 0.0)
        jw32 = sing.tile([128, 128], dt.float32, name="jw32")
        nc.vector.memset(jw32, 0.0)
        jx32 = sing.tile([128, 450], dt.float32, name="jx32")
        nc.vector.memset(jx32, 0.0)
        JP = []

        def pe_warm(n, fp32=False):
            jp = psp.tile([128, 450], dt.float32, name=f"jp{len(JP)}",
                          tag="ps")
            JP.append(jp)
            for i in range(n):
                if fp32:
                    nc.tensor.matmul(jp, jw32, jx32, start=True, stop=True,
                                     skip_group_check=True)
                else:
                    nc.tensor.matmul(jp, jw, jx,
                                     perf_mode=mybir.MatmulPerfMode.DoubleRow,
                                     start=True, stop=True,
                                     skip_group_check=True)

        def load_sign_x(im, ch):
            xc = chkp.tile([128, SP], dt.float32, name=f"px{ch}_{im}",
                           tag="chk")
            nc.sync.dma_start(out=xc, in_=px_d[im, ch * 128:(ch + 1) * 128])
            nc.scalar.activation(
                xbt[:, ch, MARG + im * SP:MARG + (im + 1) * SP], xc, AF.Sign)

        # ---- conv macro -------------------------------------------------
        def conv_half(cv, m, S, st, inject=None):
            tiles = [(im, b) for im in range(n_img) for b in (0, 1)]
            for im in range(n_img):
                S[(m, im)] = s1p.tile([128, HW], dt.float32,
                                      name=f"S{cv}_{m}_{im}", tag="s1")
            for gi, g0 in enumerate(range(0, len(tiles), 4)):
                grp = tiles[g0:g0 + 4]
                pts = {}
                for (im, b) in grp:
                    pts[(im, b)] = psp.tile([128, 450], dt.float32,
                                            name=f"cp{cv}_{m}_{im}_{b}",
                                            tag="ps")
                for d in range(9):
                    w_ap = wt_ap(cv, d, m)
                    for (im, b) in grp:
                        o = MARG + im * SP + b * 450 + _off(d)
                        nc.tensor.matmul(
                            pts[(im, b)], w_ap, xbt[:, :, o:o + 450],
                            perf_mode=mybir.MatmulPerfMode.DoubleRow,
                            start=(d == 0), stop=(d == 8))
                for (im, b) in grp:
                    pt = pts[(im, b)]
                    s_t = S[(m, im)]
                    pv = pt.rearrange("p (r c) -> p r c", c=PW)
                    sv = s_t.rearrange("p (r c) -> p r c", c=W)
                    r0 = 1 - b
                    nc.scalar.copy(sv[:, b * 14:(b + 1) * 14, :],
                                   pv[:, r0:r0 + 14, 1:29])
                    if b == 1:
                        for q in (0, 1):
                            nc.vector.bn_stats(st[m][:, im, q],
                                               s_t[:, q * 392:(q + 1) * 392])
                if inject is not None:
                    inject(gi)

        # ---- per-half stat helpers (AG round trip on the ACT queue) ----
        def half_sums(stm, tag):
            mv = cf(f"mv{tag}", 2)
            nc.vector.bn_aggr(mv, stm.rearrange("p a b s -> p (a b) s"))
            s2 = cf(f"s2{tag}", 2)
            nc.vector.tensor_scalar_mul(s2[:, 0:1], mv[:, 0:1], NLOC)
            t0 = cf(f"t0{tag}")
            nc.vector.tensor_mul(t0, mv[:, 0:1], mv[:, 0:1])
            nc.vector.tensor_add(t0, t0, mv[:, 1:2])
            nc.vector.tensor_scalar_mul(s2[:, 1:2], t0, NLOC)
            return s2

        def ag_reduce(s2, tag):
            di = dccp.tile([256], dt.float32, name=f"di{tag}", tag=f"di{tag}")
            do = dccp.tile([n_cores * 256], dt.float32, name=f"do{tag}",
                           tag=f"do{tag}")
            nc.sync.dma_start(out=di.rearrange("(c f) -> c f", f=2), in_=s2)
            nc.gpsimd.collective_compute(
                "AllGather", ALU.bypass, replica_groups=rg, ins=[di],
                outs=[do])
            g8 = cf(f"g8{tag}", 2 * n_cores)
            nc.sync.dma_start(
                out=g8.rearrange("p (f r) -> p f r", f=2),
                in_=do.rearrange("(r c f) -> c f r", c=128, f=2))
            g2 = cf(f"g2{tag}", 2)
            nc.vector.reduce_sum(g2, g8.rearrange("p (f r) -> p f r", f=2),
                                 axis=AX.X)
            return g2

        def mean_var(g2, tag):
            mean = cf(f"mean{tag}")
            var = cf(f"var{tag}")
            msq = cf(f"msq{tag}")
            nc.vector.tensor_scalar_mul(mean, g2[:, 0:1], 1.0 / NTOT)
            nc.vector.tensor_scalar_mul(var, g2[:, 1:2], 1.0 / NTOT)
            nc.vector.tensor_mul(msq, mean, mean)
            nc.vector.tensor_sub(var, var, msq)
            return mean, var

        def inv_of(var, jg, ch, tag):
            sd = cf(f"sd{tag}")
            nc.scalar.activation(sd, var, AF.Sqrt, bias=epst)
            rc = cf(f"rc{tag}")
            nc.vector.reciprocal(rc, sd)
            inv = cf(f"inv{tag}")
            nc.vector.tensor_mul(inv, rc, P(jg, ch))
            return inv

        def bn_conv_coefs(cv, g2, ch, jg, jb, jgam, tag):
            # stats of RAW S (alpha cancels through BN): c=inv,
            # dg = b - mean*inv - gamma
            mean, var = mean_var(g2, tag)
            inv = inv_of(var, jg, ch, tag)
            my = cf(f"my{tag}")
            nc.vector.tensor_mul(my, mean, inv)
            dg = cf(f"dg{tag}")
            nc.vector.tensor_sub(dg, P(jb, ch), my)
            nc.vector.tensor_sub(dg, dg, P(jgam, ch))
            return inv, dg

        def bn_plain_coefs(g2, ch, jg, jb, tag):
            mean, var = mean_var(g2, tag)
            inv = inv_of(var, jg, ch, tag)
            d = cf(f"d{tag}")
            nc.vector.tensor_mul(mean, mean, inv)
            nc.vector.tensor_sub(d, P(jb, ch), mean)
            return inv, d

        # ---- h-path emitters -------------------------------------------
        HSW = {}
        XU = {}

        def xu_dma(ch, im):
            xc = chkp.tile([128, HW], dt.float32, name=f"xu{ch}_{im}",
                           tag="chk")
            XU[(ch, im)] = xc
            nc.sync.dma_start(out=xc, in_=xu_d[im, ch * 128:(ch + 1) * 128])

        def combine1(ch, im, coefs, pool_add=False):
            # h' = prelu(c1*S+d1g) + x, fp32 in place; stats for BN3.
            c1, d1g = coefs
            s_t = S1[(ch, im)]
            t = tmpp.tile([128, HW], dt.float32, name=f"t1_{ch}_{im}",
                          tag="t")
            nc.scalar.activation(t, s_t, AF.Prelu, bias=d1g, scale=c1,
                                 alpha=P("beta1", ch))
            xc = XU.pop((ch, im))
            eng = nc.gpsimd if (pool_add or im % 2) else nc.vector
            eng.tensor_tensor(s_t, t, xc, ALU.add)
            for q in (0, 1):
                nc.vector.bn_stats(sth[ch][:, im, q],
                                   s_t[:, q * 392:(q + 1) * 392])

        def swap_out(ch, im):
            dr = dswp.tile([128, HW], dt.float32, name=f"hs{ch}_{im}",
                           tag="swap")
            HSW[(ch, im)] = dr
            nc.sync.dma_start(out=dr, in_=S1[(ch, im)])

        def sign3(ch, im):
            c3, d3 = cc3[ch]
            s_t = S1[(ch, im)]
            base = MARG + im * SP
            dst = (xbt[:, ch, base:base + SP]
                   .rearrange("p (h w) -> p h w", w=PW)[:, 1:29, 1:29])
            nc.scalar.activation(dst, s_t.rearrange("p (h w) -> p h w", w=W),
                                 AF.Sign, bias=d3, scale=c3)

        HC = {}

        def hc_dma(ch, im):
            hc = chkp.tile([128, HW], dt.float32, name=f"hc{ch}_{im}",
                           tag="chk")
            HC[(ch, im)] = hc
            nc.sync.dma_start(out=hc, in_=HSW[(ch, im)])

        def combine2(ch, im, coefs, pool_add=False):
            c2, d2g = coefs
            s2t = S2[(ch, im)]
            t2 = tmpp.tile([128, HW], dt.float32, name=f"t2_{ch}_{im}",
                           tag="t")
            nc.scalar.activation(t2, s2t, AF.Prelu, bias=d2g, scale=c2,
                                 alpha=P("beta2", ch))
            hc = HC.pop((ch, im))
            eng = nc.gpsimd if (pool_add or im % 2) else nc.vector
            eng.tensor_tensor(s2t, t2, hc, ALU.add)
            for q in (0, 1):
                nc.vector.bn_stats(stf[ch][:, im, q],
                                   s2t[:, q * 392:(q + 1) * 392])

        def bn4_out(ch, im, coefs):
            c4, d4 = coefs
            s2t = S2[(ch, im)]
            ot = otp.tile([128, HW], dt.float16, name=f"ot{ch}_{im}",
                          tag="ot")
            if im % 2:
                nc.scalar.activation(ot, s2t, AF.Identity, bias=d4, scale=c4)
            else:
                nc.vector.tensor_scalar(ot, s2t, c4, d4, ALU.mult, ALU.add)
            nc.sync.dma_start(out=o_d[im, ch * 128:(ch + 1) * 128], in_=ot)

        # ---- stats tiles (tag-shared slots, used in stage order) -------
        def st_tile(name):
            return stp.tile([128, n_img, 2, 6], dt.float32, name=name,
                            tag="st", bufs=4)

        st1 = {m: st_tile(f"st1_{m}") for m in (0, 1)}
        sth = {ch: st_tile(f"sth_{ch}") for ch in (0, 1)}
        st2 = {m: st_tile(f"st2_{m}") for m in (0, 1)}
        stf = {ch: st_tile(f"stf_{ch}") for ch in (0, 1)}
        S1 = {}
        S2 = {}
        cc3 = {}

        # ---- phase 0 + conv1 m=0 ---------------------------------------
        pe_warm(50)
        loaded = [0]

        def feed_x(upto):
            while loaded[0] < min(upto, n_img):
                for ch in (0, 1):
                    load_sign_x(loaded[0], ch)
                loaded[0] += 1

        feed_x(3)

        def inj_m0(gi):
            feed_x(5 + 2 * gi)

        conv_half(1, 0, S1, st1, inject=inj_m0)
        feed_x(n_img)
        read_warm()
        g2_10 = ag_reduce(half_sums(st1[0], "b10"), "b10")

        # ---- conv1 m=1: combine ch0 (x loads pace the SP queue), then
        # BN3-ch0; sign3-ch0 mostly lands at gap start under AG-b11.
        cc1_0 = [None]
        pa0 = [0]
        sg0 = [0]

        def run_ch0(a_upto, s_upto):
            if cc1_0[0] is None:
                cc1_0[0] = bn_conv_coefs(1, g2_10, 0, "g1", "b1", "gamma1",
                                         "b10")
                xu_dma(0, 0)
                xu_dma(0, 1)
            while pa0[0] < min(a_upto, n_img):
                im = pa0[0]
                if im + 2 < n_img:
                    xu_dma(0, im + 2)
                combine1(0, im, cc1_0[0], pool_add=True)
                pa0[0] += 1
            if pa0[0] == n_img and 0 not in cc3:
                g2h0 = ag_reduce(half_sums(sth[0], "b30"), "b30")
                cc3[0] = bn_plain_coefs(g2h0, 0, "g3", "b3", "b30")
            if 0 in cc3:
                while sg0[0] < min(s_upto, n_img):
                    sign3(0, sg0[0])
                    sg0[0] += 1

        def inj_m1(gi):
            if gi >= 1:
                run_ch0(6 * gi, 4 * (gi - 5))

        conv_half(1, 1, S1, st1, inject=inj_m1)
        pe_warm(45, fp32=True)
        # critical path first: BN1-ch1 stats AG, then ch0 leftovers (the
        # b30 AG + sign3-ch0 have slack until after b31)
        g2_11 = ag_reduce(half_sums(st1[1], "b11"), "b11")
        run_ch0(n_img, n_img)

        # ---- gap: ch1 combine + BN3-ch1 + sign3-ch1 --------------------
        cc1_1 = bn_conv_coefs(1, g2_11, 1, "g1", "b1", "gamma1", "b11")
        xu_dma(1, 0)
        xu_dma(1, 1)
        for im in range(n_img):
            if im + 2 < n_img:
                xu_dma(1, im + 2)
            combine1(1, im, cc1_1)
        g2h1 = ag_reduce(half_sums(sth[1], "b31"), "b31")
        cc3[1] = bn_plain_coefs(g2h1, 1, "g3", "b3", "b31")
        for im in range(n_img):
            sign3(1, im)

        # h' swap-outs stream during conv2-m0 (SP is otherwise idle there;
        # each must land before conv2 reuses that S tile slot).
        for im in range(n_img):
            swap_out(0, im)
        for im in range(n_img):
            swap_out(1, im)

        # ---- conv2 ------------------------------------------------------
        conv_half(2, 0, S2, st2)
        g2_20 = ag_reduce(half_sums(st2[0], "b20"), "b20")

        def bn2_coefs(ch, g2c, tag):
            c2, d2g = bn_conv_coefs(2, g2c, ch, "g2", "b2", "gamma2", tag)
            r3 = cf(f"r3{ch}")
            nc.vector.reciprocal(r3, cc3[ch][0])
            nc.vector.tensor_mul(c2, c2, r3)
            nc.vector.tensor_mul(d2g, d2g, r3)
            return c2, d2g

        cc2_0 = [None]
        cc4_0 = [None]
        done2 = [0]
        out0 = [0]

        def run2_ch0(upto, out_upto):
            if cc2_0[0] is None:
                cc2_0[0] = bn2_coefs(0, g2_20, "b20")
                hc_dma(0, 0)
                hc_dma(0, 1)
            while done2[0] < min(upto, n_img):
                im = done2[0]
                if im + 2 < n_img:
                    hc_dma(0, im + 2)
                combine2(0, im, cc2_0[0], pool_add=True)
                done2[0] += 1
            if done2[0] == n_img and cc4_0[0] is None:
                g2f0 = ag_reduce(half_sums(stf[0], "b40"), "b40")
                cc4_0[0] = bn_plain_coefs(g2f0, 0, "g4", "b4", "b40")
            if cc4_0[0] is not None:
                while out0[0] < min(out_upto, n_img):
                    bn4_out(0, out0[0], cc4_0[0])
                    out0[0] += 1

        def inj2_m1(gi):
            if gi >= 1:
                run2_ch0(6 * gi, 4 * (gi - 4))

        conv_half(2, 1, S2, st2, inject=inj2_m1)
        # critical path first: BN2-ch1 stats AG jumps ahead of the bn4-ch0
        # output drain (which only has to finish by kernel end)
        g2_21 = ag_reduce(half_sums(st2[1], "b21"), "b21")
        run2_ch0(n_img, n_img)

        # ---- tail: ch1 --------------------------------------------------
        cc2_1 = bn2_coefs(1, g2_21, "b21")
        hc_dma(1, 0)
        hc_dma(1, 1)
        for im in range(n_img):
            if im + 2 < n_img:
                hc_dma(1, im + 2)
            combine2(1, im, cc2_1)
        g2f1 = ag_reduce(half_sums(stf[1], "b41"), "b41")
        cc4_1 = bn_plain_coefs(g2f1, 1, "g4", "b4", "b41")
        for im in range(n_img):
            bn4_out(1, im, cc4_1)

    nc.compile()
    return nc


_NC_CACHE = {}


def get_nc(n_img, n_cores):
    key = (n_img, n_cores)
    if key not in _NC_CACHE:
        _NC_CACHE[key] = build_nc(n_img, n_cores)
    return _NC_CACHE[key]


def pack_pars(inputs):
    return np.stack([np.asarray(inputs[k], np.float32) for k in
                     ["g1", "b1", "g2", "b2", "g3", "b3", "g4", "b4",
                      "gamma1", "beta1", "gamma2", "beta2"]])


def make_in_maps(inputs, n_cores=8):
    x = np.asarray(inputs["x"], np.float32)
    n_img = x.shape[0] // n_cores
    pars = pack_pars(inputs)
    import ml_dtypes
    wts = []
    for k in ("w1", "w2"):
        w = np.asarray(inputs[k], np.float32)
        wts.append(np.transpose(w.reshape(C, 2, 128, 9),
                                (2, 1, 3, 0)).reshape(128, 2 * 9 * C))
    wt = np.ascontiguousarray(np.stack(wts).astype(ml_dtypes.bfloat16))
    xu = np.ascontiguousarray(x.reshape(-1, C, HW))
    xp = np.pad(x.reshape(-1, C, H, W), ((0, 0), (0, 0), (1, 1), (1, 1)))
    xp = np.ascontiguousarray(xp.reshape(-1, C, SP))
    return [
        {"px": np.ascontiguousarray(xp[c * n_img:(c + 1) * n_img]),
         "xu": np.ascontiguousarray(xu[c * n_img:(c + 1) * n_img]),
         "wt": wt, "pars": pars}
        for c in range(n_cores)
    ], n_img


def kernel(**inputs):
    from concourse.bass_utils import run_bass_kernel_spmd

    n_cores = 8
    in_maps, n_img = make_in_maps(inputs, n_cores)
    nc = get_nc(n_img, n_cores)
    res = run_bass_kernel_spmd(nc, in_maps, core_ids=list(range(n_cores)))
    return np.concatenate([res.results[c]["out"] for c in range(n_cores)],
                          axis=0).reshape(-1, C, H, W).astype(np.float32)


if __name__ == "__main__":
    nc = build_nc(2, 2)
    print("built ok")
